# revision 72
# baseline (speedup 1.0000x reference)
"""AdaptiveFrequencyAsymmetricHuberLoss on 8 TRN2 NeuronCores (Bass/Tile).

loss = mean( wf(t) * asym(t, sign(e)) * huber(e, delta(t)) ),  e = p - t
  delta(t)   = 5 + 0.05 t
  w_under(t) = 1 + 0.05 t
  w_over(t)  = 2 exp(-t/10)
  wf(t)      = clip(3 / (freq[t] + 1), 1, 3)   (t integer 0..130)
  huber      = 0.5 cl (2e - cl), cl = clip(e, -delta, delta)   (exact identity)

Sharding: pure data parallel; each of the 8 cores streams a contiguous
1/8 of the elements as [128, 16384], DMA-cast f32->bf16 on load.

Per-tile pipeline:
  ACT:  nd = -delta,  ws = w_over (Exp)
  DVE:  e  = p - t                                   [bf16 2x]
        sh = |cl| * (2e - cl) = sign(e) * 2*huber    [8-op custom, 1x]
        shp = max(sh, 0), rm = max(-sh, 0)           [bf16 4x]
        wu = 1 + 0.05 t                              [bf16 4x]
        qo = shp * ws,  qu = rm * wu                 [bf16 2x]
  PE:   ones-colsum matmuls accumulate sum(qo)+sum(qu) into one
        [1,512] PSUM bank across all tiles (only the total matters).
Host divides by 2N and reduces in float64.

The freq table is handled host-side: wf >= 1 always, and wf > 1 only
for freq counts < 2, so the host enumerates the (usually zero) table
entries with wf > 1 and the kernel adds masked correction passes per
entry (accum_out into a separate SBUF accumulator).
"""

import contextlib

import numpy as np

import concourse.bass as bass
import concourse.dve_ops as dve_ops_mod
import concourse.tile as tile
from concourse import bacc, mybir
from concourse.bass_utils import run_bass_kernel_spmd
from concourse.dve_ops import DveOp
from concourse.dve_spec import (
    Spec,
    Src0,
    Src1,
    Zero,
    _has_src1,
    lower,
    maxx,
    minn,
)
from concourse.dve_uop import DveOpSpec

N = 16_777_216
NCORES = 8
P = 128
PER_CORE = N // NCORES          # 2_097_152
FREE = PER_CORE // P            # 16384
TILE_FS = [1024, 3072, 4096, 4096, 3072, 1024]
assert sum(TILE_FS) == FREE

LN2 = 0.6931471805599453

f32 = mybir.dt.float32
bf16 = mybir.dt.bfloat16


def _register_op(name, spec):
    for o in dve_ops_mod.OPS:
        if o.name == name:
            return o
    opcode = max(dve_ops_mod._SUB_OPCODE_FOR_NAME.values()) + 1
    assert opcode < 0x20, "custom-DVE opcode rows exhausted"
    shas = {}
    for ver in ("v3", "v4"):
        try:
            c = DveOpSpec(
                name=name, opcode=opcode, uops=lower(spec, ver=ver),
                rd1_en=_has_src1(spec),
            )
            shas[ver] = c.sha(ver)
        except Exception:
            pass
    op = DveOp(name, spec, subdim=False, uops_sha=shas)
    dve_ops_mod.OPS.append(op)
    dve_ops_mod.CUSTOM_DVE_SPECS[name] = spec
    dve_ops_mod._SUB_OPCODE_FOR_NAME[name] = opcode
    return op


def _huber_signed_ref(in0, in1, c0, c1, c2):
    e = in0.astype(np.float32)
    nd = in1.astype(np.float32)
    cl = np.minimum(np.maximum(e, nd), -nd)
    return (np.abs(cl) * ((e + e) - cl)).astype(np.float32)


# sh = |cl| * (2e - cl) = sign(e) * 2*huber(e, delta);  in0 = e, in1 = -delta
_dd = Zero - Src1
_cl = minn(maxx(Src0, Src1), _dd)
_v = (Src0 + Src0) - _cl
_acl = maxx(_cl, Zero - _cl)
HUBER_SIGNED_SPEC = Spec(
    body=_acl * _v,
    reference=_huber_signed_ref,
)

HUBER_SIGNED_OP = _register_op("HUBER_SIGNED_LOSS_ANT", HUBER_SIGNED_SPEC)


def build(corrections):
    """Build + compile the SPMD graph. corrections: tuple of (k, wf_k - 1)."""
    Alu = mybir.AluOpType
    Act = mybir.ActivationFunctionType

    nc = bacc.Bacc(
        "TRN2", target_bir_lowering=False, debug=False, num_devices=NCORES
    )

    ones = nc.const_aps.aps[(bf16, 1.0)]  # [128,1] bf16 ones (matmul lhsT)

    p_ap = nc.dram_tensor("p", [P, FREE], bf16, kind="ExternalInput").ap()
    t_ap = nc.dram_tensor("tn", [P, FREE], bf16, kind="ExternalInput").ap()
    # chunks of 512: 0 = sum(shm) [sum(qo) w/ corr], 1 = sum(shm*tn)
    # [sum(qu) w/ corr]
    o_ap = nc.dram_tensor("out", [1, 1024], f32, kind="ExternalOutput").ap()
    # per-partition per-tile sum(relu(sh*ws)) via the ACT pass's accum_out
    or_ap = nc.dram_tensor(
        "outq", [P, len(TILE_FS)], f32, kind="ExternalOutput"
    ).ap()
    oc_ap = None
    if corrections:
        oc_ap = nc.dram_tensor(
            "outc", [P, len(TILE_FS) * len(corrections)], f32,
            kind="ExternalOutput",
        ).ap()

    n_mms = {0: 0, 1: 0, 2: 0}
    total_mms = sum(f // 512 for f in TILE_FS)

    with contextlib.ExitStack() as es:
        tc = es.enter_context(tile.TileContext(nc))
        io_pool = es.enter_context(tc.tile_pool(name="io", bufs=3))
        tmp = es.enter_context(tc.tile_pool(name="tmp", bufs=2))
        ps_pool = es.enter_context(
            tc.tile_pool(name="ps", bufs=1, space=bass.MemorySpace.PSUM)
        )
        acc_pool = es.enter_context(tc.tile_pool(name="acc", bufs=1))

        psrows = [
            ps_pool.tile([1, 512], f32, tag=f"ps{r}", name=f"ps{r}")
            for r in range(2)
        ]
        qacc = acc_pool.tile([P, len(TILE_FS)], f32, tag="qacc")
        accs = None
        if corrections:
            accs = acc_pool.tile([P, len(TILE_FS) * len(corrections)], f32)

        def colsum(src_ap, tf, row):
            for c in range(0, tf, 512):
                nc.tensor.matmul(
                    psrows[row][:], ones, src_ap[:, c : c + 512],
                    start=(n_mms[row] == 0),
                    stop=(n_mms[row] == total_mms - 1),
                )
                n_mms[row] += 1

        pend_qor = []

        def _emit_qor(shw_t, tf, ti):
            qo = tmp.tile([P, tf], bf16, tag="qo", name="qo")
            nc.scalar.activation(
                qo[:], shw_t[:], Act.Relu, bias=0.0, scale=1.0,
                accum_out=qacc[:, ti : ti + 1],
            )
            return qo

        col = 0
        off = 0
        for i, TF in enumerate(TILE_FS):
            sl = slice(off, off + TF)
            off += TF
            pt = io_pool.tile([P, TF], bf16, tag="pt")
            nc.sync.dma_start(out=pt[:], in_=p_ap[:, sl])
            tt = io_pool.tile([P, TF], bf16, tag="tt")  # tn = -t
            nc.sync.dma_start(out=tt[:], in_=t_ap[:, sl])

            nd = tmp.tile([P, TF], bf16, tag="nd")  # -delta = -5 - 0.05 t
            nc.scalar.activation(nd[:], tt[:], Act.Copy, bias=-5.0, scale=0.05)
            ws = tmp.tile([P, TF], bf16, tag="ws")  # w_over / 2 = exp(-0.1 t)
            nc.scalar.activation(ws[:], tt[:], Act.Exp, bias=0.0, scale=0.1)
            wu = None
            if corrections:
                wu = tmp.tile([P, TF], bf16, tag="wu")  # w_under = 1 + 0.05 t
                nc.vector.tensor_scalar(
                    out=wu[:], in0=tt[:], scalar1=-0.05, scalar2=1.0,
                    op0=Alu.mult, op1=Alu.add,
                )

            e = tmp.tile([P, TF], bf16, tag="e")  # e = p + tn = p - t
            nc.vector.tensor_tensor(out=e[:], in0=pt[:], in1=tt[:], op=Alu.add)
            sh = tmp.tile([P, TF], bf16, tag="sh")  # sign(e) * 2*huber
            nc.vector._custom_dve(HUBER_SIGNED_OP, out=sh[:], in0=e[:], in1=nd[:])
            # over side: qo = relu(sh)*ws = relu(sh*ws)  (ws > 0); the ACT
            # relu pass's accum_out delivers sum(qo) for free.  The relu
            # runs one tile behind (pending list) so the next tile's
            # nd/ws come first in ACT program order and the DVE never
            # waits on an ACT op that itself waits on the DVE.
            shw = tmp.tile([P, TF], bf16, tag="shw", bufs=3)
            nc.vector.tensor_tensor(out=shw[:], in0=sh[:], in1=ws[:], op=Alu.mult)
            if corrections:
                qo = _emit_qor(shw, TF, i)
            else:
                pend_qor.append((shw, TF, i))
                if len(pend_qor) > 1:
                    _emit_qor(*pend_qor.pop(0))
            if not corrections:
                # shm = min(sh,0) = -2*huber where e<0
                # under-sum = 0.5*(-sum(shm) + 0.05*sum(shm*tn))
                shm = tmp.tile([P, TF], bf16, tag="shm")
                nc.vector.tensor_scalar(
                    out=shm[:], in0=sh[:], scalar1=0.0, scalar2=None,
                    op0=Alu.min,
                )
                colsum(shm, TF, 0)
                rmt = tmp.tile([P, TF], bf16, tag="rmt")
                nc.vector.tensor_tensor(out=rmt[:], in0=shm[:], in1=tt[:], op=Alu.mult)
                colsum(rmt, TF, 1)
            else:
                rm = tmp.tile([P, TF], bf16, tag="rm")  # 2*huber where e<0
                nc.vector.tensor_scalar(
                    out=rm[:], in0=sh[:], scalar1=-1.0, scalar2=0.0,
                    op0=Alu.mult, op1=Alu.max,
                )
                qu = tmp.tile([P, TF], bf16, tag="qu")
                nc.vector.tensor_tensor(out=qu[:], in0=rm[:], in1=wu[:], op=Alu.mult)
                colsum(qu, TF, 0)
                colsum(qu, TF, 1)  # row1 duplicates row0; host ignores it

            for k, dw in corrections:
                # per-element loss (x2): 2*qo + qu
                qd = tmp.tile([P, TF], bf16, tag="qd")
                nc.vector.scalar_tensor_tensor(
                    out=qd[:], in0=qo[:], scalar=2.0, in1=qu[:],
                    op0=Alu.mult, op1=Alu.add,
                )
                ck = tmp.tile([P, TF], bf16, tag="ck")
                nc.vector.tensor_scalar(
                    out=ck[:], in0=tt[:], scalar1=-float(k), scalar2=None,
                    op0=Alu.is_equal,
                )
                qc = tmp.tile([P, TF], bf16, tag="qc")
                nc.vector.scalar_tensor_tensor(
                    out=qc[:], in0=ck[:], scalar=float(dw), in1=qd[:],
                    op0=Alu.mult, op1=Alu.mult,
                    accum_out=accs[:, col : col + 1],
                )
                col += 1
        while pend_qor:
            _emit_qor(*pend_qor.pop(0))
        osb = acc_pool.tile([1, 1024], f32, tag="osb")
        for r in range(2):
            nc.vector.tensor_copy(
                out=osb[:, r * 512 : (r + 1) * 512], in_=psrows[r][:]
            )
        nc.sync.dma_start(out=o_ap[:], in_=osb[:])
        nc.sync.dma_start(out=or_ap[:], in_=qacc[:])
        if corrections:
            nc.sync.dma_start(out=oc_ap[:], in_=accs[:])
    nc.compile()
    return nc


_cache = {}


def get_nc(corrections):
    key = tuple(corrections)
    if key not in _cache:
        _cache[key] = build(key)
    return _cache[key]


def make_in_maps(predictions, targets):
    import ml_dtypes

    # The kernel computes in bf16 either way (the previous version
    # DMA-cast f32->bf16 on load with identical round-to-nearest);
    # converting on the host is numerically identical and halves the
    # bytes DMA'd. Targets are integers 0..130: exact in bf16.
    p = np.ascontiguousarray(
        np.asarray(predictions, dtype=np.float32).astype(ml_dtypes.bfloat16)
    ).reshape(NCORES, P, FREE)
    tn = np.ascontiguousarray(
        (-np.asarray(targets, dtype=np.float32)).astype(ml_dtypes.bfloat16)
    ).reshape(NCORES, P, FREE)
    return [{"p": p[c], "tn": tn[c]} for c in range(NCORES)]


def freq_corrections(freq_counts):
    fc = np.asarray(freq_counts, dtype=np.float32)
    wf = np.clip(
        np.float32(3.0) / (fc + np.float32(1.0)), np.float32(1.0), np.float32(3.0)
    )
    ks = np.nonzero(wf > 1.0)[0]
    return tuple((int(k), float(wf[k] - 1.0)) for k in ks)


def _run(in_maps, corrections, **kwargs):
    nc = get_nc(corrections)
    return run_bass_kernel_spmd(nc, in_maps, core_ids=list(range(NCORES)), **kwargs)


def reduce_results(res, corrections):
    total = np.float64(0.0)
    for c in range(NCORES):
        o = np.asarray(res.results[c]["out"], dtype=np.float64).reshape(2, 512)
        sqo = np.asarray(res.results[c]["outq"], dtype=np.float64).sum()
        if corrections:
            # outq = sum(qo); chunk0 = sum(qu)
            total += sqo + 0.5 * o[0].sum()
            total += 0.5 * np.asarray(
                res.results[c]["outc"], dtype=np.float64
            ).sum()
        else:
            # outq = sum(qo); chunk0 = sum(shm); chunk1 = sum(shm*tn)
            total += sqo + 0.5 * (-o[0].sum() + 0.05 * o[1].sum())
    return np.array(total / N, dtype=np.float32)


def kernel(predictions, targets, freq_counts):
    corrections = freq_corrections(freq_counts)
    in_maps = make_in_maps(predictions, targets)
    res = _run(in_maps, corrections)
    return reduce_results(res, corrections)


# revision 76
# speedup vs baseline: 1.0152x; 1.0152x over previous
"""AdaptiveFrequencyAsymmetricHuberLoss on 8 TRN2 NeuronCores (Bass/Tile).

loss = mean( wf(t) * asym(t, sign(e)) * huber(e, delta(t)) ),  e = p - t
  delta(t)   = 5 + 0.05 t
  w_under(t) = 1 + 0.05 t
  w_over(t)  = 2 exp(-t/10)
  wf(t)      = clip(3 / (freq[t] + 1), 1, 3)   (t integer 0..130)
  huber      = 0.5 cl (2e - cl), cl = clip(e, -delta, delta)   (exact identity)

Sharding: pure data parallel; each of the 8 cores streams a contiguous
1/8 of the elements as [128, 16384], DMA-cast f32->bf16 on load.

Per-tile pipeline:
  ACT:  nd = -delta,  ws = w_over (Exp)
  DVE:  e  = p - t                                   [bf16 2x]
        sh = |cl| * (2e - cl) = sign(e) * 2*huber    [8-op custom, 1x]
        shp = max(sh, 0), rm = max(-sh, 0)           [bf16 4x]
        wu = 1 + 0.05 t                              [bf16 4x]
        qo = shp * ws,  qu = rm * wu                 [bf16 2x]
  PE:   ones-colsum matmuls accumulate sum(qo)+sum(qu) into one
        [1,512] PSUM bank across all tiles (only the total matters).
Host divides by 2N and reduces in float64.

The freq table is handled host-side: wf >= 1 always, and wf > 1 only
for freq counts < 2, so the host enumerates the (usually zero) table
entries with wf > 1 and the kernel adds masked correction passes per
entry (accum_out into a separate SBUF accumulator).
"""

import contextlib

import numpy as np

import concourse.bass as bass
import concourse.dve_ops as dve_ops_mod
import concourse.tile as tile
from concourse import bacc, mybir
from concourse.bass_utils import run_bass_kernel_spmd
from concourse.dve_ops import DveOp
from concourse.dve_spec import (
    Spec,
    Src0,
    Src1,
    Zero,
    _has_src1,
    lower,
    maxx,
    minn,
)
from concourse.dve_uop import DveOpSpec

N = 16_777_216
NCORES = 8
P = 128
PER_CORE = N // NCORES          # 2_097_152
FREE = PER_CORE // P            # 16384
TILE_FS = [1024, 3072, 4096, 4096, 3072, 1024]
assert sum(TILE_FS) == FREE

LN2 = 0.6931471805599453

f32 = mybir.dt.float32
bf16 = mybir.dt.bfloat16


def _register_op(name, spec):
    for o in dve_ops_mod.OPS:
        if o.name == name:
            return o
    opcode = max(dve_ops_mod._SUB_OPCODE_FOR_NAME.values()) + 1
    assert opcode < 0x20, "custom-DVE opcode rows exhausted"
    shas = {}
    for ver in ("v3", "v4"):
        try:
            c = DveOpSpec(
                name=name, opcode=opcode, uops=lower(spec, ver=ver),
                rd1_en=_has_src1(spec),
            )
            shas[ver] = c.sha(ver)
        except Exception:
            pass
    op = DveOp(name, spec, subdim=False, uops_sha=shas)
    dve_ops_mod.OPS.append(op)
    dve_ops_mod.CUSTOM_DVE_SPECS[name] = spec
    dve_ops_mod._SUB_OPCODE_FOR_NAME[name] = opcode
    return op


def _huber_signed_ref(in0, in1, c0, c1, c2):
    e = in0.astype(np.float32)
    nd = in1.astype(np.float32)
    cl = np.minimum(np.maximum(e, nd), -nd)
    return (np.abs(cl) * ((e + e) - cl)).astype(np.float32)


# sh = |cl| * (2e - cl) = sign(e) * 2*huber(e, delta);  in0 = e, in1 = -delta
_dd = Zero - Src1
_cl = minn(maxx(Src0, Src1), _dd)
_v = (Src0 + Src0) - _cl
_acl = maxx(_cl, Zero - _cl)
HUBER_SIGNED_SPEC = Spec(
    body=_acl * _v,
    reference=_huber_signed_ref,
)

HUBER_SIGNED_OP = _register_op("HUBER_SIGNED_LOSS_ANT", HUBER_SIGNED_SPEC)


def build(corrections):
    """Build + compile the SPMD graph. corrections: tuple of (k, wf_k - 1)."""
    Alu = mybir.AluOpType
    Act = mybir.ActivationFunctionType

    nc = bacc.Bacc(
        "TRN2", target_bir_lowering=False, debug=False, num_devices=NCORES
    )

    ones = nc.const_aps.aps[(bf16, 1.0)]  # [128,1] bf16 ones (matmul lhsT)

    p_ap = nc.dram_tensor("p", [P, FREE], bf16, kind="ExternalInput").ap()
    t_ap = nc.dram_tensor("tn", [P, FREE], bf16, kind="ExternalInput").ap()
    # chunks of 512: 0 = sum(shm) [sum(qo) w/ corr], 1 = sum(shm*tn)
    # [sum(qu) w/ corr]
    o_ap = nc.dram_tensor("out", [1, 1024], f32, kind="ExternalOutput").ap()
    # per-partition per-tile sum(relu(sh*ws)) via the ACT pass's accum_out
    or_ap = nc.dram_tensor(
        "outq", [P, len(TILE_FS)], f32, kind="ExternalOutput"
    ).ap()
    oc_ap = None
    if corrections:
        oc_ap = nc.dram_tensor(
            "outc", [P, len(TILE_FS) * len(corrections)], f32,
            kind="ExternalOutput",
        ).ap()

    n_mms = {0: 0, 1: 0, 2: 0}
    total_mms = sum(f // 512 for f in TILE_FS)

    with contextlib.ExitStack() as es:
        tc = es.enter_context(tile.TileContext(nc))
        io_pool = es.enter_context(tc.tile_pool(name="io", bufs=3))
        tmp = es.enter_context(tc.tile_pool(name="tmp", bufs=2))
        ps_pool = es.enter_context(
            tc.tile_pool(name="ps", bufs=1, space=bass.MemorySpace.PSUM)
        )
        acc_pool = es.enter_context(tc.tile_pool(name="acc", bufs=1))

        psrows = [
            ps_pool.tile([1, 512], f32, tag=f"ps{r}", name=f"ps{r}")
            for r in range(2)
        ]
        qacc = acc_pool.tile([P, len(TILE_FS)], f32, tag="qacc")
        accs = None
        if corrections:
            accs = acc_pool.tile([P, len(TILE_FS) * len(corrections)], f32)

        def colsum(src_ap, tf, row):
            for c in range(0, tf, 512):
                nc.tensor.matmul(
                    psrows[row][:], ones, src_ap[:, c : c + 512],
                    start=(n_mms[row] == 0),
                    stop=(n_mms[row] == total_mms - 1),
                )
                n_mms[row] += 1

        pend_qor = []

        def _emit_qor(shw_t, tf, ti):
            qo = tmp.tile([P, tf], bf16, tag="qo", name="qo")
            nc.scalar.activation(
                qo[:], shw_t[:], Act.Relu, bias=0.0, scale=1.0,
                accum_out=qacc[:, ti : ti + 1],
            )
            return qo

        col = 0
        off = 0
        for i, TF in enumerate(TILE_FS):
            sl = slice(off, off + TF)
            off += TF
            pt = io_pool.tile([P, TF], bf16, tag="pt")
            nc.sync.dma_start(out=pt[:], in_=p_ap[:, sl])
            tt = io_pool.tile([P, TF], bf16, tag="tt")  # tn = -t
            nc.sync.dma_start(out=tt[:], in_=t_ap[:, sl])

            nd = tmp.tile([P, TF], bf16, tag="nd")  # -delta = -5 - 0.05 t
            nc.scalar.activation(nd[:], tt[:], Act.Copy, bias=-5.0, scale=0.05)
            ws = tmp.tile([P, TF], bf16, tag="ws")  # w_over / 2 = exp(-0.1 t)
            nc.scalar.activation(ws[:], tt[:], Act.Exp, bias=0.0, scale=0.1)
            wu = None
            if corrections:
                wu = tmp.tile([P, TF], bf16, tag="wu", bufs=1)  # 1 + 0.05 t
                nc.vector.tensor_scalar(
                    out=wu[:], in0=tt[:], scalar1=-0.05, scalar2=1.0,
                    op0=Alu.mult, op1=Alu.add,
                )

            e = tmp.tile([P, TF], bf16, tag="e")  # e = p + tn = p - t
            nc.vector.tensor_tensor(out=e[:], in0=pt[:], in1=tt[:], op=Alu.add)
            sh = tmp.tile([P, TF], bf16, tag="sh")  # sign(e) * 2*huber
            nc.vector._custom_dve(HUBER_SIGNED_OP, out=sh[:], in0=e[:], in1=nd[:])
            # over side: qo = relu(sh)*ws = relu(sh*ws)  (ws > 0); the ACT
            # relu pass's accum_out delivers sum(qo) for free.  The relu
            # runs one tile behind (pending list) so the next tile's
            # nd/ws come first in ACT program order and the DVE never
            # waits on an ACT op that itself waits on the DVE.
            shw = tmp.tile([P, TF], bf16, tag="shw", bufs=3 if not corrections else 1)
            nc.vector.tensor_tensor(out=shw[:], in0=sh[:], in1=ws[:], op=Alu.mult)
            if corrections:
                qo = _emit_qor(shw, TF, i)
            else:
                pend_qor.append((shw, TF, i))
                if len(pend_qor) > 1:
                    _emit_qor(*pend_qor.pop(0))
            if not corrections:
                # shm = min(sh,0) = -2*huber where e<0
                # under-sum = 0.5*(-sum(shm) + 0.05*sum(shm*tn))
                shm = tmp.tile([P, TF], bf16, tag="shm")
                nc.vector.tensor_scalar(
                    out=shm[:], in0=sh[:], scalar1=0.0, scalar2=None,
                    op0=Alu.min,
                )
                colsum(shm, TF, 0)
                rmt = tmp.tile([P, TF], bf16, tag="rmt")
                nc.vector.tensor_tensor(out=rmt[:], in0=shm[:], in1=tt[:], op=Alu.mult)
                colsum(rmt, TF, 1)
            else:
                rm = tmp.tile([P, TF], bf16, tag="rm", bufs=1)  # 2h where e<0
                nc.vector.tensor_scalar(
                    out=rm[:], in0=sh[:], scalar1=-1.0, scalar2=0.0,
                    op0=Alu.mult, op1=Alu.max,
                )
                qu = tmp.tile([P, TF], bf16, tag="qu", bufs=1)
                nc.vector.tensor_tensor(out=qu[:], in0=rm[:], in1=wu[:], op=Alu.mult)
                colsum(qu, TF, 0)
                colsum(qu, TF, 1)  # row1 duplicates row0; host ignores it

            for k, dw in corrections:
                # per-element loss (x2): 2*qo + qu
                qd = tmp.tile([P, TF], bf16, tag="qd", bufs=1)
                nc.vector.scalar_tensor_tensor(
                    out=qd[:], in0=qo[:], scalar=2.0, in1=qu[:],
                    op0=Alu.mult, op1=Alu.add,
                )
                ck = tmp.tile([P, TF], bf16, tag="ck", bufs=1)
                nc.vector.tensor_scalar(
                    out=ck[:], in0=tt[:], scalar1=-float(k), scalar2=None,
                    op0=Alu.is_equal,
                )
                qc = tmp.tile([P, TF], bf16, tag="qc", bufs=1)
                nc.vector.scalar_tensor_tensor(
                    out=qc[:], in0=ck[:], scalar=float(dw), in1=qd[:],
                    op0=Alu.mult, op1=Alu.mult,
                    accum_out=accs[:, col : col + 1],
                )
                col += 1
        while pend_qor:
            _emit_qor(*pend_qor.pop(0))
        osb = acc_pool.tile([1, 1024], f32, tag="osb")
        for r in range(2):
            nc.vector.tensor_copy(
                out=osb[:, r * 512 : (r + 1) * 512], in_=psrows[r][:]
            )
        nc.sync.dma_start(out=o_ap[:], in_=osb[:])
        nc.sync.dma_start(out=or_ap[:], in_=qacc[:])
        if corrections:
            nc.sync.dma_start(out=oc_ap[:], in_=accs[:])
    nc.compile()
    return nc


_cache = {}


def get_nc(corrections):
    key = tuple(corrections)
    if key not in _cache:
        _cache[key] = build(key)
    return _cache[key]


def make_in_maps(predictions, targets):
    import ml_dtypes

    # The kernel computes in bf16 either way (the previous version
    # DMA-cast f32->bf16 on load with identical round-to-nearest);
    # converting on the host is numerically identical and halves the
    # bytes DMA'd. Targets are integers 0..130: exact in bf16.
    p = np.ascontiguousarray(
        np.asarray(predictions, dtype=np.float32).astype(ml_dtypes.bfloat16)
    ).reshape(NCORES, P, FREE)
    tn = np.ascontiguousarray(
        (-np.asarray(targets, dtype=np.float32)).astype(ml_dtypes.bfloat16)
    ).reshape(NCORES, P, FREE)
    return [{"p": p[c], "tn": tn[c]} for c in range(NCORES)]


def freq_corrections(freq_counts):
    fc = np.asarray(freq_counts, dtype=np.float32)
    wf = np.clip(
        np.float32(3.0) / (fc + np.float32(1.0)), np.float32(1.0), np.float32(3.0)
    )
    ks = np.nonzero(wf > 1.0)[0]
    return tuple((int(k), float(wf[k] - 1.0)) for k in ks)


def _run(in_maps, corrections, **kwargs):
    nc = get_nc(corrections)
    return run_bass_kernel_spmd(nc, in_maps, core_ids=list(range(NCORES)), **kwargs)


def reduce_results(res, corrections):
    total = np.float64(0.0)
    for c in range(NCORES):
        o = np.asarray(res.results[c]["out"], dtype=np.float64).reshape(2, 512)
        sqo = np.asarray(res.results[c]["outq"], dtype=np.float64).sum()
        if corrections:
            # outq = sum(qo); chunk0 = sum(qu)
            total += sqo + 0.5 * o[0].sum()
            total += 0.5 * np.asarray(
                res.results[c]["outc"], dtype=np.float64
            ).sum()
        else:
            # outq = sum(qo); chunk0 = sum(shm); chunk1 = sum(shm*tn)
            total += sqo + 0.5 * (-o[0].sum() + 0.05 * o[1].sum())
    return np.array(total / N, dtype=np.float32)


def kernel(predictions, targets, freq_counts):
    corrections = freq_corrections(freq_counts)
    in_maps = make_in_maps(predictions, targets)
    res = _run(in_maps, corrections)
    return reduce_results(res, corrections)
